# revision 118
# baseline (speedup 1.0000x reference)
"""Distributed Trainium2 Bass kernel for nn_AnyAttention (sparse attention).

Sharding: 8 cores = 2 batches (data-parallel) x 4 head-groups (tensor-parallel,
4 heads / 256 channels each). Attention never crosses head shards; each core
returns its partial row-parallel projection output [C, Lq] (bf16) and the host
does the standard TP unshard (sum the 4 partials per batch) plus the final
transpose. b_proj rides on the hg==0 cores only.

Key structure (v2):
 - Host prep: masked-out K columns dropped + padded to Lkp (pad bias -1e30),
   positional adds (q+qpos, k+kpos) folded host-side, LN gamma folded into
   the projection weights host-side (beta enters as a host-computed W@beta
   row consumed by a device-side rank-1), everything shipped C-major bf16 so
   all contractions have C on partitions.
 - LayerNorm stats via token-stationary matmuls: lhsT = x-tile [128c,128t],
   rhs = ones column -> PSUM [128t, 1] accumulated over the 8 c-tiles; the
   sum-of-squares column likewise from DVE-squared tiles. mu/var/rstd live
   in COLUMN layout [128t, n_tiles], which is exactly what the exp
   per-partition scale (k), the v4 eviction scale (v), and PV want. The few
   rows needed (negmu and u=colsum(W) for the rank-1 LN mean-corrections,
   and the q-side rstd broadcast) are transposed column->row ON THE PE via
   identity matmuls (output free size 128) instead of DRAM bounces, which
   would otherwise queue behind the big input transfers on the shared DMA
   path. The q-side rstd row is partition-broadcast on the idle GPSIMD.
 - rstd = exp(-0.5*ln(var+eps)) on ACT: Ln/Exp/Square/Copy all live in the
   natural_log_exp_and_others table, so the kernel needs exactly ONE
   activation-table load. The k-side rstd folds the softmax SCALE via the
   exp bias (ln SCALE).
 - Scores S^T[k,q] per (q-half, dt) with two heads at partition bases 0/64;
   exp on ACT with the mask bias and k-side rstd*SCALE folded in.
 - PV runs q-stationary: lhsT = E-tile [128k, 128q], rhs = v4a [128k, 65]
   (65th column = softmax denominator) -> PSUM [128q, 65]; the eviction
   multiplies by the per-partition reciprocal denominator, and an SBUF->SBUF
   DMA transpose ([128,128] bf16 tiles) restores the C-major layout for the
   output projection. This halves PV's PE cost vs the [65, 512q] orientation.
 - Output written bf16; host sums the 4 TP partials per batch in f32.
"""

import math
import os
import numpy as np

import concourse.bass as bass
import concourse.tile as tile
from concourse import bacc, mybir
from concourse.bass_utils import run_bass_kernel_spmd

# The axon trace path imports antenv.axon_hooks; stub it if absent so a
# BASS_TRACE env var in the calling environment degrades gracefully.
try:
    import antenv.axon_hooks  # noqa: F401
except ImportError:
    import sys as _sys
    import types as _types
    _m = _types.ModuleType("antenv.axon_hooks")
    _m.get_axon_ntff_profile_hook = lambda: None
    _sys.modules["antenv.axon_hooks"] = _m

F32 = mybir.dt.float32
BF16 = mybir.dt.bfloat16
AF = mybir.ActivationFunctionType

B = 2
LQ = 1024
LK = 2048
C = 1024
G = 16
HPC = 4          # heads per core
HC = 256         # head channels per core
CH = 64          # channels per head
SCALE = (C / G) ** -0.5   # 0.125
EPS = 1e-5
NCT = C // 128   # number of C tiles (8)
NDT = C // 128   # number of output-d tiles (8)
NQT = LQ // 128  # number of q token tiles (8)

LAST_EXEC_NS = None
LAST_RESULTS = None
_NC_CACHE = {}


def _slices(total, step):
    out = []
    o = 0
    while o < total:
        s = min(step, total - o)
        out.append((o, s))
        o += s
    return out


def _compile_pinned(nc, keep="natural_log_exp_and_others"):
    """Compile with the act-table chooser pinned to one table.

    The insertion pass greedily picks the first table containing each
    activation function, which thrashes between `natural_log` and
    `exp_and_others` when Ln and Exp interleave. Blanking the contents of
    every other table (names and indices unchanged, so the emitted
    act_func_set_id still references the real json entry) makes it settle
    on the one table that contains Ln, Exp, Square, and Copy. Restored
    immediately after compile.
    """
    import concourse.bacc as bacc_mod
    orig = bacc_mod.get_activation_tables

    def patched(arch):
        tabs = orig(arch)
        return {name: (s if name == keep else set()) for name, s in tabs.items()}

    bacc_mod.get_activation_tables = patched
    try:
        nc.compile()
    finally:
        bacc_mod.get_activation_tables = orig


def build_nc(Lkp, ln_identity=True, bproj_zero=True):
    NKT = Lkp // 128
    nc = bacc.Bacc(None, target_bir_lowering=False, debug=False)

    # ---- I/O (per-core shards) ----
    qT = nc.dram_tensor("qT", [C, LQ], BF16, kind="ExternalInput")
    kT = nc.dram_tensor("kT", [C, Lkp], BF16, kind="ExternalInput")
    vT = nc.dram_tensor("vT", [C, Lkp], BF16, kind="ExternalInput")
    wqT = nc.dram_tensor("wqT", [C, HC], BF16, kind="ExternalInput")
    wkT = nc.dram_tensor("wkT", [C, HC], BF16, kind="ExternalInput")
    wvT = nc.dram_tensor("wvT", [C, HC], BF16, kind="ExternalInput")
    wp = nc.dram_tensor("wp", [128, HPC // 2, C], BF16, kind="ExternalInput")
    madd = nc.dram_tensor("madd", [128, NKT], F32, kind="ExternalInput")
    eye = nc.dram_tensor("eye", [128, 128], BF16, kind="ExternalInput")
    if not ln_identity:
        # host-computed W @ beta rows (the LN beta term after the gamma fold)
        bwq = nc.dram_tensor("bwq", [1, HC], BF16, kind="ExternalInput")
        bwk = nc.dram_tensor("bwk", [1, HC], BF16, kind="ExternalInput")
        bwv = nc.dram_tensor("bwv", [1, HC], BF16, kind="ExternalInput")
    if not bproj_zero:
        bproj = nc.dram_tensor("bproj", [128, NDT], F32, kind="ExternalInput")
    out = nc.dram_tensor("out", [C, LQ], BF16, kind="ExternalOutput")

    with tile.TileContext(nc) as tc:
        with (
            tc.tile_pool(name="persist", bufs=1) as P,
            tc.tile_pool(name="rows", bufs=1) as R,
            tc.tile_pool(name="rows2", bufs=2) as R2,
            tc.tile_pool(name="sq", bufs=3) as SQ,
            tc.tile_pool(name="psA", bufs=2, space="PSUM") as PSA,
            tc.tile_pool(name="psS", bufs=3, space="PSUM") as PSS,
            tc.tile_pool(name="psPV", bufs=2, space="PSUM") as PSPV,
            tc.tile_pool(name="psT", bufs=1, space="PSUM") as PST,
        ):
            ones_col = P.tile([128, 1], BF16, tag="ones_col", name="ones_col")
            nc.vector.memset(ones_col, 1.0 / C)
            ones1_col = P.tile([128, 1], BF16, tag="ones1_col", name="ones1_col")
            nc.vector.memset(ones1_col, 1.0)
            eps_col = P.tile([128, 1], F32, tag="eps_col", name="eps_col")
            nc.vector.memset(eps_col, EPS)
            lnsc_col = P.tile([128, 1], F32, tag="lnsc_col", name="lnsc_col")
            nc.vector.memset(lnsc_col, math.log(SCALE))
            # one persistent stats bank for all three tensors' column chains
            # (accumulate-only onto an explicit zero fill; start_tensor_calc
            # would wipe the whole 2KB bank row on real HW)
            ps_st = PST.tile([128, 512], F32, tag="stats", name="stats")
            nc.vector.memset(ps_st[:, :], 0.0)

            sqp_ctx = tc.tile_pool(name="sqp", bufs=1)
            SQP = sqp_ctx.__enter__()

            # ---- all load DMAs up front, in transfer-priority order ----
            eye_sb = P.tile([128, 128], BF16, tag="eye", name="eye")
            nc.sync.dma_start(out=eye_sb, in_=eye[:, :])
            madd_sb = P.tile([128, NKT], F32, tag="madd", name="madd")
            nc.sync.dma_start(out=madd_sb, in_=madd[:, :])
            bw_sb = {}
            if not ln_identity:
                for nm, bwd in (("q", bwq), ("k", bwk), ("v", bwv)):
                    bw_sb[nm] = R.tile([1, HC], BF16, tag=f"bw_{nm}", name=f"bw_{nm}")
                    nc.sync.dma_start(out=bw_sb[nm], in_=bwd[:, :])
            if not bproj_zero:
                bproj_sb = P.tile([128, NDT], F32, tag="bproj", name="bproj")
                nc.sync.dma_start(out=bproj_sb, in_=bproj[:, :])

            x_bf = {}
            sq_of = {}
            # q next (its stats chain is the longest-lead PE work); squares
            # split across ACT (idle pre-exp) and DVE
            xq = P.tile([128, NCT, LQ], BF16, tag="x_q", name="x_q")
            x_bf["q"] = xq
            sqq = SQP.tile([128, NCT, LQ], BF16, tag="sq_q", name="sq_q")
            sq_of["q"] = sqq
            qT_r = qT.rearrange("(j p) t -> p j t", p=128)
            for jj in range(0, NCT, 2):
                nc.sync.dma_start(out=xq[:, jj:jj + 2, :], in_=qT_r[:, jj:jj + 2, :])
                nc.vector.tensor_mul(sqq[:, jj, :], xq[:, jj, :], xq[:, jj, :])
                nc.vector.tensor_mul(sqq[:, jj + 1, :], xq[:, jj + 1, :],
                                     xq[:, jj + 1, :])

            w_bf = {}
            for nm, wd in (("q", wqT), ("k", wkT)):
                w_bf[nm] = P.tile([128, NCT, HC], BF16, tag=f"w_{nm}", name=f"w_{nm}")
                nc.sync.dma_start(out=w_bf[nm], in_=wd.rearrange("(j p) d -> p j d", p=128))

            # k chunks (squares emitted later on DVE, after the q4 evictions)
            xk = P.tile([128, NCT, Lkp], BF16, tag="x_k", name="x_k")
            x_bf["k"] = xk
            sqk = SQP.tile([128, NCT, Lkp], BF16, tag="sq_k", name="sq_k")
            sq_of["k"] = sqk
            kT_r = kT.rearrange("(j p) t -> p j t", p=128)
            for jj in range(0, NCT, 2):
                nc.sync.dma_start(out=xk[:, jj:jj + 2, :], in_=kT_r[:, jj:jj + 2, :])

            # v chunks before wv/wp so the v stats chain starts early
            xv = P.tile([128, NCT, Lkp], BF16, tag="x_v", name="x_v")
            x_bf["v"] = xv
            sqv = SQP.tile([128, NCT, Lkp], BF16, tag="sq_v", name="sq_v")
            sq_of["v"] = sqv
            vT_r = vT.rearrange("(j p) t -> p j t", p=128)
            for jj in range(0, NCT, 2):
                nc.sync.dma_start(out=xv[:, jj:jj + 2, :], in_=vT_r[:, jj:jj + 2, :])

            w_bf["v"] = P.tile([128, NCT, HC], BF16, tag="w_v", name="w_v")
            nc.sync.dma_start(out=w_bf["v"], in_=wvT.rearrange("(j p) d -> p j d", p=128))
            wp_bf = P.tile([128, HPC // 2, C], BF16, tag="wp", name="wp")
            nc.sync.dma_start(out=wp_bf, in_=wp[:, :, :])

            # ---- stats: token-stationary swap matmuls ----
            stat = {}

            def rstd_of(nm, scale=1.0):
                st = stat[nm]
                ntt = st["ntt"]
                r = SQ.tile([128, 16], F32, tag=f"r_{nm}", name=f"r_{nm}", bufs=1)
                if scale == 1.0:
                    nc.scalar.activation(r[:, :ntt], st["lnv"][:, :ntt], AF.Exp,
                                         scale=-0.5)
                else:
                    nc.scalar.activation(r[:, :ntt], st["lnv"][:, :ntt], AF.Exp,
                                         scale=-0.5, bias=lnsc_col[:, :])
                return r

            def emit_stats(nm, Lt):
                ntt = Lt // 128
                xb = x_bf[nm]
                sq = sq_of[nm]
                base = {"q": 0, "k": 160, "v": 320}[nm]
                ps = ps_st[:, base:base + 48]
                # accumulate-only onto the pre-zeroed persistent stats bank;
                # start_tensor_calc would wipe the whole 2KB bank row on HW.
                # skip_group_check bypasses the simulator's pairing assertion.

                def acc(out_ap, lhsT, rhs, last=False):
                    nc.tensor.matmul(out_ap, lhsT, rhs,
                                     start=False, stop=last,
                                     skip_group_check=True)

                for j in range(NCT):
                    for tt in range(ntt):
                        acc(ps[:, 2 * tt:2 * tt + 1],
                            xb[:, j, tt * 128:(tt + 1) * 128], ones_col[:, :])
                    for tt in range(ntt):
                        acc(ps[:, 2 * tt + 1:2 * tt + 2],
                            sq[:, j, tt * 128:(tt + 1) * 128], ones_col[:, :])
                ncols = 2 * ntt
                # u = colsum(W) as two [128,1] columns at cols 40:42
                for hcc in range(2):
                    for j in range(NCT):
                        acc(ps[:, 40 + hcc:41 + hcc],
                            w_bf[nm][:, j, hcc * 128:(hcc + 1) * 128],
                            ones1_col[:, :],
                            last=(hcc == 1 and j == NCT - 1))
                st = SQ.tile([128, 48], F32, tag="stcols", name="stcols", bufs=3)
                nc.vector.tensor_copy(st[:, :ncols], ps[:, :ncols])
                u_bf = R2.tile([128, 2], BF16, tag="u_bf", name="u_bf")
                nc.vector.tensor_copy(u_bf, ps[:, 40:42])
                mu = st[:, 0:ncols:2]
                msq = st[:, 1:ncols:2]
                var = SQ.tile([128, 16], F32, tag="var", name="var", bufs=3)
                nc.vector.tensor_mul(var[:, :ntt], mu, mu)
                nc.vector.tensor_sub(var[:, :ntt], msq, var[:, :ntt])
                negmu_bf = R2.tile([128, 16], BF16, tag="negmu_bf", name="negmu_bf")
                nc.vector.tensor_scalar_mul(negmu_bf[:, :ntt], mu, -1.0)
                # lnv = ln(var + eps); rstd & friends via exp(a*lnv + b)
                lnv = SQ.tile([128, 16], F32, tag="lnv", name="lnv", bufs=3)
                nc.scalar.activation(lnv[:, :ntt], var[:, :ntt], AF.Ln,
                                     bias=eps_col[:, :])
                stat[nm] = dict(lnv=lnv, ntt=ntt)

                # ---- column->row transposes on PE (identity matmuls) ----
                # Each half-row is 4 independent [1,128] writes (start+stop
                # per segment) at psum partitions 0/32/64/96 of a borrowed
                # S-pool bank; every row evicts to its own [1,512] SBUF tile
                # at base partition 0, so all rank-1 operand pairs share a
                # base and no u duplication is needed. Evictions ride on ACT
                # for q/k (idle pre-exp) and DVE for v (ACT is exp-bound).
                nhalf = -(-ntt // 4)  # 512-wide halves of the rows
                assert nhalf <= 2, f"Lkp too large for row layout: {ntt}"

                def rowcopy(dst, src, i=0):
                    # v rows on DVE (ACT is exp-bound then); q/k rows split
                    # across ACT and DVE so the copies run in parallel
                    if nm == "v" or i % 2 == 1:
                        nc.vector.tensor_copy(dst, src)
                    else:
                        nc.scalar.activation(dst, src, AF.Copy)

                jobs = []  # (sbuf_tag, col_aps)
                jobs.append((f"negmu0_{nm}",
                             [negmu_bf[:, tt:tt + 1] for tt in range(min(ntt, 4))]))
                if nhalf > 1:
                    jobs.append((f"negmu1_{nm}",
                                 [negmu_bf[:, tt:tt + 1] for tt in range(4, ntt)]))
                jobs.append((f"u_{nm}", [u_bf[:, hcc:hcc + 1] for hcc in range(2)]))
                if not ln_identity:
                    sd = SQ.tile([128, 16], F32, tag="sd", name="sd", bufs=3)
                    nc.scalar.activation(sd[:, :ntt], lnv[:, :ntt], AF.Exp,
                                         scale=0.5)
                    sd_bf = R2.tile([128, 16], BF16, tag="sd_bf", name="sd_bf")
                    nc.vector.tensor_copy(sd_bf[:, :ntt], sd[:, :ntt])
                    jobs.append((f"sd0_{nm}",
                                 [sd_bf[:, tt:tt + 1] for tt in range(min(ntt, 4))]))
                    if nhalf > 1:
                        jobs.append((f"sd1_{nm}",
                                     [sd_bf[:, tt:tt + 1] for tt in range(4, ntt)]))
                chunks = [jobs[j0:j0 + 4] for j0 in range(0, len(jobs), 4)]
                if nm == "q":
                    # rq rides in its own psum chunk so the negmu/u rows
                    # don't wait for the rstd chain
                    rq_bf = R2.tile([128, 16], BF16, tag="rq_bf", name="rq_bf")
                    rq_f = rstd_of("q")
                    nc.vector.tensor_copy(rq_bf[:, :ntt], rq_f[:, :ntt])
                    rqjobs = [("rq0", [rq_bf[:, tt:tt + 1]
                                       for tt in range(min(ntt, 4))])]
                    if nhalf > 1:
                        rqjobs.append(("rq1", [rq_bf[:, tt:tt + 1]
                                               for tt in range(4, ntt)]))
                    chunks.append(rqjobs)
                row_sb = {}
                for chunk in chunks:
                    psr = PSS.tile([128, 512], F32, tag="S", name="S")
                    for slot, (tag, cols) in enumerate(chunk):
                        pb = 32 * slot
                        for i, col_ap in enumerate(cols):
                            nc.tensor.matmul(
                                psr[pb:pb + 1, i * 128:(i + 1) * 128],
                                col_ap, eye_sb[:, :],
                                start=True, stop=True,
                                tile_position=(0, pb))
                    for slot, (tag, cols) in enumerate(chunk):
                        w = len(cols) * 128
                        rsb = R.tile([1, 512], BF16, tag=tag, name=tag)
                        rowcopy(rsb[0:1, :w], psr[32 * slot:32 * slot + 1, :w],
                                i=slot)
                        row_sb[tag] = rsb

                stat[nm]["negmu_row"] = (
                    lambda o, s, _n=nm: row_sb[f"negmu{o // 512}_{_n}"][
                        0:1, o % 512:o % 512 + s])
                stat[nm]["u_row"] = (
                    lambda po, off, s, _n=nm: row_sb[f"u_{_n}"][0:1, off:off + s])
                if not ln_identity:
                    stat[nm]["sd_row"] = (
                        lambda o, s, _n=nm: row_sb[f"sd{o // 512}_{_n}"][
                            0:1, o % 512:o % 512 + s])
                if nm == "q":
                    # broadcast the rstd row across partitions on Pool
                    rqb = P.tile([128, LQ], BF16, tag="rq_bc", name="rq_bc")
                    for h in range(nhalf):
                        nc.gpsimd.partition_broadcast(
                            rqb[:, h * 512:(h + 1) * 512],
                            row_sb[f"rq{h}"][0:1, :])
                    stat[nm]["rq_bc"] = rqb

            emit_stats("q", LQ)
            rq_bc = stat["q"]["rq_bc"]

            # all k squares as one uninterrupted DVE block (q4 evictions
            # now ride ACT+Pool, so nothing interleaves into the k path)
            for j in range(NCT):
                nc.vector.tensor_mul(sqk[:, j, :], xk[:, j, :], xk[:, j, :])

            # ---- q4 projections ----
            q4 = P.tile([128, 2, LQ], BF16, tag="q4", name="q4")
            k4 = P.tile([128, 2, Lkp], BF16, tag="k4", name="k4")

            def emit_qk4_mains(nm, dt, o, s):
                ps = PSA.tile([128, 512], F32, tag="main", name="main")
                for j in range(NCT):
                    nc.tensor.matmul(ps[:, :s], w_bf[nm][:, j, dt * 128:(dt + 1) * 128],
                                     x_bf[nm][:, j, o:o + s], start=(j == 0), stop=False)
                return ps

            def emit_qk4_fin(nm, dt, o, s, ps):
                dest, rbc = (q4, rq_bc) if nm == "q" else (k4, None)
                st = stat[nm]
                nc.tensor.matmul(ps[:, :s], st["u_row"](o, dt * 128, 128),
                                 st["negmu_row"](o, s), start=False,
                                 stop=ln_identity)
                if not ln_identity:
                    nc.tensor.matmul(ps[:, :s], bw_sb[nm][:, dt * 128:(dt + 1) * 128],
                                     st["sd_row"](o, s), start=False, stop=True)
                if rbc is not None:
                    # ACT evicts the raw psum; the rstd multiply runs on the
                    # idle GPSIMD so the DVE queue stays clear for the k path
                    qraw = SQ.tile([128, 512], BF16, tag="qraw", name="qraw")
                    nc.scalar.activation(qraw[:, :s], ps[:, :s], AF.Copy)
                    nc.gpsimd.tensor_mul(dest[:, dt, o:o + s], qraw[:, :s],
                                         rbc[:, o:o + s])
                elif dt == 0:
                    # ACT is idle pre-exp; keeps the DVE queue off the k path
                    nc.scalar.activation(dest[:, dt, o:o + s], ps[:, :s], AF.Copy)
                else:
                    nc.vector.tensor_copy(dest[:, dt, o:o + s], ps[:, :s])

            def emit_qk4(nm, dt, o, s):
                emit_qk4_fin(nm, dt, o, s, emit_qk4_mains(nm, dt, o, s))

            emit_qk4("q", 0, 0, 512)
            emit_qk4("q", 0, 512, 512)
            emit_qk4("q", 1, 0, 512)
            emit_qk4("q", 1, 512, 512)

            emit_stats("k", Lkp)
            rk_col = rstd_of("k", SCALE)

            # ---- attention pipeline ----
            groups = [(o, s, dt) for (o, s) in _slices(LQ, 512) for dt in range(2)]
            et_of = {}

            ep_ctx = tc.tile_pool(name="epool", bufs=6)
            EP = ep_ctx.__enter__()

            def emit_sexp(g, kts=None):
                (o, s, dt) = groups[g]
                if kts is None:
                    kts = range(NKT)
                if g in et_of:
                    ets = et_of[g]
                else:
                    ets = []
                    for hh in range(2):
                        et = EP.tile([128, NKT, 512], BF16, tag="E", name="E")
                        ets.append(et)
                    et_of[g] = ets
                for kt in kts:
                    pss = []
                    for hh in range(2):
                        pb = 64 * hh
                        ps = PSS.tile([128, 512], F32, tag="S", name="S")
                        pss.append(ps)
                        nc.tensor.matmul(ps[:, :s],
                                         k4[pb:pb + CH, dt, kt * 128:(kt + 1) * 128],
                                         q4[pb:pb + CH, dt, o:o + s],
                                         start=True, stop=True)
                    for hh in range(2):
                        nc.scalar.activation(ets[hh][:, kt, :s], pss[hh][:, :s],
                                             AF.Exp,
                                             bias=madd_sb[:, kt:kt + 1],
                                             scale=rk_col[:, kt:kt + 1])

            # interleave k4 eviction halves with the S/exp k-tiles they gate;
            # the v squares ride the dt0 stretch on DVE, and the v stats sit
            # just inside dt1 so Ln_v/Exp_rv land at the g0/g1 boundary of
            # the in-order ACT exp queue
            rv_col = None
            for dt in range(2):
                if dt == 1:
                    emit_qk4("k", 1, 0, min(512, Lkp))
                    emit_stats("v", Lkp)
                    rv_col = rstd_of("v")
                for i, (o, s) in enumerate(_slices(Lkp, 512)):
                    if dt != 1 or o != 0:
                        emit_qk4("k", dt, o, s)
                    if dt == 0:
                        hi = NCT if o + s >= Lkp else min(4 * i + 4, NCT)
                        for j in range(4 * i, hi):
                            nc.vector.tensor_mul(sqv[:, j, :], xv[:, j, :],
                                                 xv[:, j, :])
                    emit_sexp(dt, range(o // 128, (o + s) // 128))

            # ---- v4a: [128t, HPC*(CH+1)] per ktile; 65th col = ones ----
            v4a = P.tile([128, NKT, HPC * (CH + 1)], BF16, tag="v4a", name="v4a")

            def emit_v4a_kt(kt):
                ps = PSA.tile([128, 512], F32, tag="main", name="main")
                for j in range(NCT):
                    nc.tensor.matmul(ps[:, :HC], x_bf["v"][:, j, kt * 128:(kt + 1) * 128],
                                     w_bf["v"][:, j, :], start=(j == 0), stop=False)
                nc.tensor.matmul(ps[:, :HC],
                                 stat["v"]["negmu_row"](kt * 128, 128),
                                 stat["v"]["u_row"](kt * 128, 0, HC), start=False,
                                 stop=ln_identity)
                if not ln_identity:
                    nc.tensor.matmul(ps[:, :HC],
                                     stat["v"]["sd_row"](kt * 128, 128),
                                     bw_sb["v"][:, :], start=False, stop=True)
                nc.vector.tensor_scalar_mul(
                    v4a[:, kt, :].rearrange("p (h x) -> p h x", h=HPC)[:, :, 0:CH],
                    ps[:, 0:HC].rearrange("p (h x) -> p h x", h=HPC),
                    rv_col[:, kt:kt + 1])
                nc.vector.memset(
                    v4a[:, kt, :].rearrange("p (h x) -> p h x", h=HPC)[:, :, CH:CH + 1], 1.0)

            # oqc[q, qt, h, c]: normalized per-head attention output, q-major
            oqc = P.tile([128, NQT, HPC, CH], BF16, tag="oqc", name="oqc")
            o_sb = P.tile([128, HPC // 2, LQ], BF16, tag="o_sb", name="o_sb")

            def emit_pv_qt(g, qt):
                (o, s, dt) = groups[g]
                ets = et_of[g]
                qo = qt * 128 - o
                for hh in range(2):
                    h = 2 * dt + hh
                    et = ets[hh]
                    ppv = PSPV.tile([128, CH + 1], F32, tag="pv", name="pv")
                    for kt in range(NKT):
                        nc.tensor.matmul(ppv[:, :],
                                         et[:, kt, qo:qo + 128],
                                         v4a[:, kt, h * (CH + 1):(h + 1) * (CH + 1)],
                                         start=(kt == 0), stop=(kt == NKT - 1))
                    rcp = R2.tile([128, 1], F32, tag="rcp", name="rcp", bufs=8)
                    nc.vector.reciprocal_approx_fast(out=rcp, in_=ppv[:, CH:CH + 1])
                    nc.vector.tensor_scalar_mul(oqc[:, qt, h, :],
                                                ppv[:, 0:CH], rcp)
                if dt == 1:
                    eng = nc.scalar if g == 3 else nc.sync
                    for t in range(HPC // 2):
                        # g3 transposes issue from ACT (idle post-exp, and
                        # its SEQ has no pending out-DMA waits like SP)
                        eng.dma_start_transpose(
                            out=o_sb[:, t, qt * 128:(qt + 1) * 128],
                            in_=oqc[:, qt, 2 * t:2 * t + 2, :])
                if qt == (o + s) // 128 - 1:
                    et_of.pop(g)

            def emit_proj(o, s, tail=False):
                for dt2 in range(NDT):
                    ps = PSA.tile([128, 512], F32, tag="main", name="main")
                    for t in range(HPC // 2):
                        nc.tensor.matmul(ps[:, :s], wp_bf[:, t, dt2 * 128:(dt2 + 1) * 128],
                                         o_sb[:, t, o:o + s], start=(t == 0),
                                         stop=(t == HPC // 2 - 1))
                    ot = SQ.tile([128, 512], BF16, tag="ot", name="ot")
                    if bproj_zero and tail:
                        # halve each eviction across DVE+ACT so the PSA
                        # rotation isn't paced by one serial DVE stream
                        h = s // 2
                        nc.vector.tensor_copy(ot[:, :h], ps[:, :h])
                        nc.scalar.activation(ot[:, h:s], ps[:, h:s], AF.Copy)
                    elif bproj_zero:
                        nc.vector.tensor_copy(ot[:, :s], ps[:, :s])
                    else:
                        nc.vector.tensor_scalar_add(ot[:, :s], ps[:, :s],
                                                    bproj_sb[:, dt2:dt2 + 1])
                    nc.sync.dma_start(out=out[dt2 * 128:(dt2 + 1) * 128, o:o + s],
                                      in_=ot[:, :s])

            # ---- v4a, then drain. PV(g0)/PV(g1) run BEFORE the S(g2)/S(g3)
            # stretches: the in-order PE would otherwise sit inside the
            # exp-paced S lockstep (PSS rotation) while ready PV work waits.
            for kt in range(NKT):
                emit_v4a_kt(kt)
                if kt >= 2:
                    # feed S(g2) into the ACT exp queue while v4a runs
                    emit_sexp(2, [kt - 2])
            for qt in range(4):
                emit_pv_qt(0, qt)
            emit_sexp(2, [NKT - 2, NKT - 1])
            for qt in range(4):
                emit_pv_qt(1, qt)
            emit_sexp(3)
            emit_proj(0, 512)
            for qt in range(4, 8):
                emit_pv_qt(2, qt)
            for qt in range(4, 8):
                emit_pv_qt(3, qt)
            emit_proj(512, 512, tail=True)
            ep_ctx.__exit__(None, None, None)
            sqp_ctx.__exit__(None, None, None)

    _compile_pinned(nc)
    return nc


def prepare_in_maps(q, k, v, qpos, kpos, mask,
                    ln_q_w, ln_q_b, ln_k_w, ln_k_b, ln_v_w, ln_v_b,
                    w_q, w_k, w_v, w_proj, b_proj):
    import ml_dtypes
    bf = ml_dtypes.bfloat16
    f = np.float32
    q = np.asarray(q, f) + np.asarray(qpos, f).reshape(B, LQ, C)
    k = np.asarray(k, f) + np.asarray(kpos, f).reshape(B, LK, C)
    v = np.asarray(v, f)
    mask = np.asarray(mask)

    keeps = [np.flatnonzero(mask[b, 0, 0] == 0) for b in range(B)]
    Lkp = max(128, -(-max(len(kp) for kp in keeps) // 128) * 128)
    NKT = Lkp // 128

    def colmajor(vec, ntiles):
        return np.ascontiguousarray(vec.reshape(ntiles, 128).T.astype(f))

    ident = all(np.all(np.asarray(g) == 1.0) for g in (ln_q_w, ln_k_w, ln_v_w)) \
        and all(np.all(np.asarray(b) == 0.0) for b in (ln_q_b, ln_k_b, ln_v_b))
    bz = bool(np.all(np.asarray(b_proj) == 0.0))

    # gamma folds into W host-side; beta enters as host-computed W@beta rows
    w_eff = {}
    bw_full = {}
    for nm, w_, g_, b_ in (("q", w_q, ln_q_w, ln_q_b), ("k", w_k, ln_k_w, ln_k_b),
                           ("v", w_v, ln_v_w, ln_v_b)):
        w_ = np.asarray(w_, f)
        if ident:
            w_eff[nm] = w_
        else:
            w_eff[nm] = w_ * np.asarray(g_, f)[None, :]
            bw_full[nm] = w_ @ np.asarray(b_, f)

    in_maps = []
    for core in range(8):
        b, hg = core // 4, core % 4
        kp = keeps[b]
        nk = len(kp)
        hs = slice(hg * HC, (hg + 1) * HC)

        def padT(x2d):  # [n, C] -> [C, Lkp] bf16
            outp = np.zeros((C, Lkp), bf)
            outp[:, :x2d.shape[0]] = x2d.T.astype(bf)
            return np.ascontiguousarray(outp)

        madd_np = np.full(Lkp, -1e30, f)
        madd_np[:nk] = 0.0
        m = {
            "qT": np.ascontiguousarray(q[b].T.astype(bf)),
            "kT": padT(k[b][kp]),
            "vT": padT(v[b][kp]),
            "wqT": np.ascontiguousarray(w_eff["q"][hs, :].T.astype(bf)),
            "wkT": np.ascontiguousarray(w_eff["k"][hs, :].T.astype(bf)),
            "wvT": np.ascontiguousarray(w_eff["v"][hs, :].T.astype(bf)),
            # wp[64*(h%2)+p, h//2, d] = w_proj[d, hg*256 + 64h + p]
            "wp": np.ascontiguousarray(
                np.asarray(w_proj, f)[:, hs].T.reshape(HPC // 2, 2, CH, C)
                .transpose(1, 2, 0, 3).reshape(128, HPC // 2, C).astype(bf)),
            "madd": colmajor(madd_np, NKT),
            "eye": np.ascontiguousarray(np.eye(128).astype(bf)),
        }
        if not ident:
            m["bwq"] = np.ascontiguousarray(bw_full["q"][hs][None, :].astype(bf))
            m["bwk"] = np.ascontiguousarray(bw_full["k"][hs][None, :].astype(bf))
            m["bwv"] = np.ascontiguousarray(bw_full["v"][hs][None, :].astype(bf))
        if not bz:
            m["bproj"] = colmajor(
                np.asarray(b_proj, f) if hg == 0 else np.zeros(C, f), NDT)
        in_maps.append(m)
    return in_maps, Lkp, ident, bz


def kernel(**inputs):
    global LAST_EXEC_NS, LAST_RESULTS
    f = np.float32
    in_maps, Lkp, ident, bz = prepare_in_maps(**inputs)
    key = (Lkp, ident, bz)
    nc = _NC_CACHE.get(key)
    if nc is None:
        nc = build_nc(Lkp, ln_identity=ident, bproj_zero=bz)
        _NC_CACHE[key] = nc
    trace = os.environ.get("KERNEL_TRACE", "0") == "1"
    res = run_bass_kernel_spmd(nc, in_maps, core_ids=list(range(8)), trace=trace)
    LAST_EXEC_NS = res.exec_time_ns
    LAST_RESULTS = res

    out_full = np.zeros((B, LQ, C), f)
    for b in range(B):
        acc = np.zeros((C, LQ), f)
        for hg in range(4):
            acc += res.results[b * 4 + hg]["out"].astype(f)
        out_full[b] = acc.T
    return out_full


# revision 119
# speedup vs baseline: 1.0026x; 1.0026x over previous
"""Distributed Trainium2 Bass kernel for nn_AnyAttention (sparse attention).

Sharding: 8 cores = 2 batches (data-parallel) x 4 head-groups (tensor-parallel,
4 heads / 256 channels each). Attention never crosses head shards; each core
returns its partial row-parallel projection output [C, Lq] (bf16) and the host
does the standard TP unshard (sum the 4 partials per batch) plus the final
transpose. b_proj rides on the hg==0 cores only.

Key structure (v2):
 - Host prep: masked-out K columns dropped + padded to Lkp (pad bias -1e30),
   positional adds (q+qpos, k+kpos) folded host-side, LN gamma folded into
   the projection weights host-side (beta enters as a host-computed W@beta
   row consumed by a device-side rank-1), everything shipped C-major bf16 so
   all contractions have C on partitions.
 - LayerNorm stats via token-stationary matmuls: lhsT = x-tile [128c,128t],
   rhs = ones column -> PSUM [128t, 1] accumulated over the 8 c-tiles; the
   sum-of-squares column likewise from DVE-squared tiles. mu/var/rstd live
   in COLUMN layout [128t, n_tiles], which is exactly what the exp
   per-partition scale (k), the v4 eviction scale (v), and PV want. The few
   rows needed (negmu and u=colsum(W) for the rank-1 LN mean-corrections,
   and the q-side rstd broadcast) are transposed column->row ON THE PE via
   identity matmuls (output free size 128) instead of DRAM bounces, which
   would otherwise queue behind the big input transfers on the shared DMA
   path. The q-side rstd row is partition-broadcast on the idle GPSIMD.
 - rstd = exp(-0.5*ln(var+eps)) on ACT: Ln/Exp/Square/Copy all live in the
   natural_log_exp_and_others table, so the kernel needs exactly ONE
   activation-table load. The k-side rstd folds the softmax SCALE via the
   exp bias (ln SCALE).
 - Scores S^T[k,q] per (q-half, dt) with two heads at partition bases 0/64;
   exp on ACT with the mask bias and k-side rstd*SCALE folded in.
 - PV runs q-stationary: lhsT = E-tile [128k, 128q], rhs = v4a [128k, 65]
   (65th column = softmax denominator) -> PSUM [128q, 65]; the eviction
   multiplies by the per-partition reciprocal denominator, and an SBUF->SBUF
   DMA transpose ([128,128] bf16 tiles) restores the C-major layout for the
   output projection. This halves PV's PE cost vs the [65, 512q] orientation.
 - Output written bf16; host sums the 4 TP partials per batch in f32.
"""

import math
import os
import numpy as np

import concourse.bass as bass
import concourse.tile as tile
from concourse import bacc, mybir
from concourse.bass_utils import run_bass_kernel_spmd

# The axon trace path imports antenv.axon_hooks; stub it if absent so a
# BASS_TRACE env var in the calling environment degrades gracefully.
try:
    import antenv.axon_hooks  # noqa: F401
except ImportError:
    import sys as _sys
    import types as _types
    _m = _types.ModuleType("antenv.axon_hooks")
    _m.get_axon_ntff_profile_hook = lambda: None
    _sys.modules["antenv.axon_hooks"] = _m

F32 = mybir.dt.float32
BF16 = mybir.dt.bfloat16
AF = mybir.ActivationFunctionType

B = 2
LQ = 1024
LK = 2048
C = 1024
G = 16
HPC = 4          # heads per core
HC = 256         # head channels per core
CH = 64          # channels per head
SCALE = (C / G) ** -0.5   # 0.125
EPS = 1e-5
NCT = C // 128   # number of C tiles (8)
NDT = C // 128   # number of output-d tiles (8)
NQT = LQ // 128  # number of q token tiles (8)

LAST_EXEC_NS = None
LAST_RESULTS = None
_NC_CACHE = {}


def _slices(total, step):
    out = []
    o = 0
    while o < total:
        s = min(step, total - o)
        out.append((o, s))
        o += s
    return out


def _compile_pinned(nc, keep="natural_log_exp_and_others"):
    """Compile with the act-table chooser pinned to one table.

    The insertion pass greedily picks the first table containing each
    activation function, which thrashes between `natural_log` and
    `exp_and_others` when Ln and Exp interleave. Blanking the contents of
    every other table (names and indices unchanged, so the emitted
    act_func_set_id still references the real json entry) makes it settle
    on the one table that contains Ln, Exp, Square, and Copy. Restored
    immediately after compile.
    """
    import concourse.bacc as bacc_mod
    orig = bacc_mod.get_activation_tables

    def patched(arch):
        tabs = orig(arch)
        return {name: (s if name == keep else set()) for name, s in tabs.items()}

    bacc_mod.get_activation_tables = patched
    try:
        nc.compile()
    finally:
        bacc_mod.get_activation_tables = orig


def build_nc(Lkp, ln_identity=True, bproj_zero=True):
    NKT = Lkp // 128
    nc = bacc.Bacc(None, target_bir_lowering=False, debug=False)

    # ---- I/O (per-core shards) ----
    qT = nc.dram_tensor("qT", [C, LQ], BF16, kind="ExternalInput")
    kT = nc.dram_tensor("kT", [C, Lkp], BF16, kind="ExternalInput")
    vT = nc.dram_tensor("vT", [C, Lkp], BF16, kind="ExternalInput")
    wqT = nc.dram_tensor("wqT", [C, HC], BF16, kind="ExternalInput")
    wkT = nc.dram_tensor("wkT", [C, HC], BF16, kind="ExternalInput")
    wvT = nc.dram_tensor("wvT", [C, HC], BF16, kind="ExternalInput")
    wp = nc.dram_tensor("wp", [128, HPC // 2, C], BF16, kind="ExternalInput")
    madd = nc.dram_tensor("madd", [128, NKT], F32, kind="ExternalInput")
    eye = nc.dram_tensor("eye", [128, 128], BF16, kind="ExternalInput")
    if not ln_identity:
        # host-computed W @ beta rows (the LN beta term after the gamma fold)
        bwq = nc.dram_tensor("bwq", [1, HC], BF16, kind="ExternalInput")
        bwk = nc.dram_tensor("bwk", [1, HC], BF16, kind="ExternalInput")
        bwv = nc.dram_tensor("bwv", [1, HC], BF16, kind="ExternalInput")
    if not bproj_zero:
        bproj = nc.dram_tensor("bproj", [128, NDT], F32, kind="ExternalInput")
    out = nc.dram_tensor("out", [C, LQ], BF16, kind="ExternalOutput")

    with tile.TileContext(nc) as tc:
        with (
            tc.tile_pool(name="persist", bufs=1) as P,
            tc.tile_pool(name="rows", bufs=1) as R,
            tc.tile_pool(name="rows2", bufs=2) as R2,
            tc.tile_pool(name="sq", bufs=3) as SQ,
            tc.tile_pool(name="psA", bufs=2, space="PSUM") as PSA,
            tc.tile_pool(name="psS", bufs=3, space="PSUM") as PSS,
            tc.tile_pool(name="psPV", bufs=2, space="PSUM") as PSPV,
            tc.tile_pool(name="psT", bufs=1, space="PSUM") as PST,
        ):
            ones_col = P.tile([128, 1], BF16, tag="ones_col", name="ones_col")
            nc.vector.memset(ones_col, 1.0 / C)
            ones1_col = P.tile([128, 1], BF16, tag="ones1_col", name="ones1_col")
            nc.vector.memset(ones1_col, 1.0)
            eps_col = P.tile([128, 1], F32, tag="eps_col", name="eps_col")
            nc.vector.memset(eps_col, EPS)
            lnsc_col = P.tile([128, 1], F32, tag="lnsc_col", name="lnsc_col")
            nc.vector.memset(lnsc_col, math.log(SCALE))
            # one persistent stats bank for all three tensors' column chains
            # (accumulate-only onto an explicit zero fill; start_tensor_calc
            # would wipe the whole 2KB bank row on real HW)
            ps_st = PST.tile([128, 512], F32, tag="stats", name="stats")
            nc.vector.memset(ps_st[:, :], 0.0)

            sqp_ctx = tc.tile_pool(name="sqp", bufs=1)
            SQP = sqp_ctx.__enter__()

            # ---- all load DMAs up front, in transfer-priority order ----
            eye_sb = P.tile([128, 128], BF16, tag="eye", name="eye")
            nc.sync.dma_start(out=eye_sb, in_=eye[:, :])
            madd_sb = P.tile([128, NKT], F32, tag="madd", name="madd")
            nc.sync.dma_start(out=madd_sb, in_=madd[:, :])
            bw_sb = {}
            if not ln_identity:
                for nm, bwd in (("q", bwq), ("k", bwk), ("v", bwv)):
                    bw_sb[nm] = R.tile([1, HC], BF16, tag=f"bw_{nm}", name=f"bw_{nm}")
                    nc.sync.dma_start(out=bw_sb[nm], in_=bwd[:, :])
            if not bproj_zero:
                bproj_sb = P.tile([128, NDT], F32, tag="bproj", name="bproj")
                nc.sync.dma_start(out=bproj_sb, in_=bproj[:, :])

            x_bf = {}
            sq_of = {}
            # q next (its stats chain is the longest-lead PE work); squares
            # split across ACT (idle pre-exp) and DVE
            xq = P.tile([128, NCT, LQ], BF16, tag="x_q", name="x_q")
            x_bf["q"] = xq
            sqq = SQP.tile([128, NCT, LQ], BF16, tag="sq_q", name="sq_q")
            sq_of["q"] = sqq
            qT_r = qT.rearrange("(j p) t -> p j t", p=128)
            for jj in range(0, NCT, 2):
                nc.sync.dma_start(out=xq[:, jj:jj + 2, :], in_=qT_r[:, jj:jj + 2, :])
                nc.vector.tensor_mul(sqq[:, jj, :], xq[:, jj, :], xq[:, jj, :])
                nc.vector.tensor_mul(sqq[:, jj + 1, :], xq[:, jj + 1, :],
                                     xq[:, jj + 1, :])

            w_bf = {}
            for nm, wd in (("q", wqT), ("k", wkT)):
                w_bf[nm] = P.tile([128, NCT, HC], BF16, tag=f"w_{nm}", name=f"w_{nm}")
                nc.sync.dma_start(out=w_bf[nm], in_=wd.rearrange("(j p) d -> p j d", p=128))

            # k chunks (squares emitted later on DVE, after the q4 evictions)
            xk = P.tile([128, NCT, Lkp], BF16, tag="x_k", name="x_k")
            x_bf["k"] = xk
            sqk = SQP.tile([128, NCT, Lkp], BF16, tag="sq_k", name="sq_k")
            sq_of["k"] = sqk
            kT_r = kT.rearrange("(j p) t -> p j t", p=128)
            for jj in range(0, NCT, 2):
                nc.sync.dma_start(out=xk[:, jj:jj + 2, :], in_=kT_r[:, jj:jj + 2, :])

            # v chunks before wv/wp so the v stats chain starts early
            xv = P.tile([128, NCT, Lkp], BF16, tag="x_v", name="x_v")
            x_bf["v"] = xv
            sqv = SQP.tile([128, NCT, Lkp], BF16, tag="sq_v", name="sq_v")
            sq_of["v"] = sqv
            vT_r = vT.rearrange("(j p) t -> p j t", p=128)
            for jj in range(0, NCT, 2):
                nc.sync.dma_start(out=xv[:, jj:jj + 2, :], in_=vT_r[:, jj:jj + 2, :])

            w_bf["v"] = P.tile([128, NCT, HC], BF16, tag="w_v", name="w_v")
            nc.sync.dma_start(out=w_bf["v"], in_=wvT.rearrange("(j p) d -> p j d", p=128))
            wp_bf = P.tile([128, HPC // 2, C], BF16, tag="wp", name="wp")
            nc.sync.dma_start(out=wp_bf, in_=wp[:, :, :])

            # ---- stats: token-stationary swap matmuls ----
            stat = {}

            def rstd_of(nm, scale=1.0):
                st = stat[nm]
                ntt = st["ntt"]
                r = SQ.tile([128, 16], F32, tag=f"r_{nm}", name=f"r_{nm}", bufs=1)
                if scale == 1.0:
                    nc.scalar.activation(r[:, :ntt], st["lnv"][:, :ntt], AF.Exp,
                                         scale=-0.5)
                else:
                    nc.scalar.activation(r[:, :ntt], st["lnv"][:, :ntt], AF.Exp,
                                         scale=-0.5, bias=lnsc_col[:, :])
                return r

            def emit_stats(nm, Lt):
                ntt = Lt // 128
                xb = x_bf[nm]
                sq = sq_of[nm]
                base = {"q": 0, "k": 160, "v": 320}[nm]
                ps = ps_st[:, base:base + 48]
                # accumulate-only onto the pre-zeroed persistent stats bank;
                # start_tensor_calc would wipe the whole 2KB bank row on HW.
                # skip_group_check bypasses the simulator's pairing assertion.

                def acc(out_ap, lhsT, rhs, last=False):
                    nc.tensor.matmul(out_ap, lhsT, rhs,
                                     start=False, stop=last,
                                     skip_group_check=True)

                for j in range(NCT):
                    for tt in range(ntt):
                        acc(ps[:, 2 * tt:2 * tt + 1],
                            xb[:, j, tt * 128:(tt + 1) * 128], ones_col[:, :])
                    for tt in range(ntt):
                        acc(ps[:, 2 * tt + 1:2 * tt + 2],
                            sq[:, j, tt * 128:(tt + 1) * 128], ones_col[:, :])
                ncols = 2 * ntt
                # u = colsum(W) as two [128,1] columns at cols 40:42
                for hcc in range(2):
                    for j in range(NCT):
                        acc(ps[:, 40 + hcc:41 + hcc],
                            w_bf[nm][:, j, hcc * 128:(hcc + 1) * 128],
                            ones1_col[:, :],
                            last=(hcc == 1 and j == NCT - 1))
                st = SQ.tile([128, 48], F32, tag="stcols", name="stcols", bufs=3)
                nc.vector.tensor_copy(st[:, :ncols], ps[:, :ncols])
                u_bf = R2.tile([128, 2], BF16, tag="u_bf", name="u_bf")
                nc.vector.tensor_copy(u_bf, ps[:, 40:42])
                mu = st[:, 0:ncols:2]
                msq = st[:, 1:ncols:2]
                var = SQ.tile([128, 16], F32, tag="var", name="var", bufs=3)
                nc.vector.tensor_mul(var[:, :ntt], mu, mu)
                nc.vector.tensor_sub(var[:, :ntt], msq, var[:, :ntt])
                negmu_bf = R2.tile([128, 16], BF16, tag="negmu_bf", name="negmu_bf")
                nc.vector.tensor_scalar_mul(negmu_bf[:, :ntt], mu, -1.0)
                # lnv = ln(var + eps); rstd & friends via exp(a*lnv + b)
                lnv = SQ.tile([128, 16], F32, tag="lnv", name="lnv", bufs=3)
                nc.scalar.activation(lnv[:, :ntt], var[:, :ntt], AF.Ln,
                                     bias=eps_col[:, :])
                stat[nm] = dict(lnv=lnv, ntt=ntt)

                # ---- column->row transposes on PE (identity matmuls) ----
                # Each half-row is 4 independent [1,128] writes (start+stop
                # per segment) at psum partitions 0/32/64/96 of a borrowed
                # S-pool bank; every row evicts to its own [1,512] SBUF tile
                # at base partition 0, so all rank-1 operand pairs share a
                # base and no u duplication is needed. Evictions ride on ACT
                # for q/k (idle pre-exp) and DVE for v (ACT is exp-bound).
                nhalf = -(-ntt // 4)  # 512-wide halves of the rows
                assert nhalf <= 2, f"Lkp too large for row layout: {ntt}"

                def rowcopy(dst, src, i=0):
                    # v rows on DVE (ACT is exp-bound then); q/k rows split
                    # across ACT and DVE so the copies run in parallel
                    if nm == "v" or i % 2 == 1:
                        nc.vector.tensor_copy(dst, src)
                    else:
                        nc.scalar.activation(dst, src, AF.Copy)

                jobs = []  # (sbuf_tag, col_aps)
                jobs.append((f"negmu0_{nm}",
                             [negmu_bf[:, tt:tt + 1] for tt in range(min(ntt, 4))]))
                if nhalf > 1:
                    jobs.append((f"negmu1_{nm}",
                                 [negmu_bf[:, tt:tt + 1] for tt in range(4, ntt)]))
                jobs.append((f"u_{nm}", [u_bf[:, hcc:hcc + 1] for hcc in range(2)]))
                if not ln_identity:
                    sd = SQ.tile([128, 16], F32, tag="sd", name="sd", bufs=3)
                    nc.scalar.activation(sd[:, :ntt], lnv[:, :ntt], AF.Exp,
                                         scale=0.5)
                    sd_bf = R2.tile([128, 16], BF16, tag="sd_bf", name="sd_bf")
                    nc.vector.tensor_copy(sd_bf[:, :ntt], sd[:, :ntt])
                    jobs.append((f"sd0_{nm}",
                                 [sd_bf[:, tt:tt + 1] for tt in range(min(ntt, 4))]))
                    if nhalf > 1:
                        jobs.append((f"sd1_{nm}",
                                     [sd_bf[:, tt:tt + 1] for tt in range(4, ntt)]))
                chunks = [jobs[j0:j0 + 4] for j0 in range(0, len(jobs), 4)]
                if nm == "q":
                    # rq rides in its own psum chunk so the negmu/u rows
                    # don't wait for the rstd chain
                    rq_bf = R2.tile([128, 16], BF16, tag="rq_bf", name="rq_bf")
                    rq_f = rstd_of("q")
                    nc.vector.tensor_copy(rq_bf[:, :ntt], rq_f[:, :ntt])
                    rqjobs = [("rq0", [rq_bf[:, tt:tt + 1]
                                       for tt in range(min(ntt, 4))])]
                    if nhalf > 1:
                        rqjobs.append(("rq1", [rq_bf[:, tt:tt + 1]
                                               for tt in range(4, ntt)]))
                    chunks.append(rqjobs)
                row_sb = {}
                for chunk in chunks:
                    psr = PSS.tile([128, 512], F32, tag="S", name="S")
                    for slot, (tag, cols) in enumerate(chunk):
                        pb = 32 * slot
                        for i, col_ap in enumerate(cols):
                            nc.tensor.matmul(
                                psr[pb:pb + 1, i * 128:(i + 1) * 128],
                                col_ap, eye_sb[:, :],
                                start=True, stop=True,
                                tile_position=(0, pb))
                    for slot, (tag, cols) in enumerate(chunk):
                        w = len(cols) * 128
                        rsb = R.tile([1, 512], BF16, tag=tag, name=tag)
                        rowcopy(rsb[0:1, :w], psr[32 * slot:32 * slot + 1, :w],
                                i=slot)
                        row_sb[tag] = rsb

                stat[nm]["negmu_row"] = (
                    lambda o, s, _n=nm: row_sb[f"negmu{o // 512}_{_n}"][
                        0:1, o % 512:o % 512 + s])
                stat[nm]["u_row"] = (
                    lambda po, off, s, _n=nm: row_sb[f"u_{_n}"][0:1, off:off + s])
                if not ln_identity:
                    stat[nm]["sd_row"] = (
                        lambda o, s, _n=nm: row_sb[f"sd{o // 512}_{_n}"][
                            0:1, o % 512:o % 512 + s])
                if nm == "q":
                    # broadcast the rstd row across partitions on Pool
                    rqb = P.tile([128, LQ], BF16, tag="rq_bc", name="rq_bc")
                    for h in range(nhalf):
                        nc.gpsimd.partition_broadcast(
                            rqb[:, h * 512:(h + 1) * 512],
                            row_sb[f"rq{h}"][0:1, :])
                    stat[nm]["rq_bc"] = rqb

            emit_stats("q", LQ)
            rq_bc = stat["q"]["rq_bc"]

            # all k squares as one uninterrupted DVE block (q4 evictions
            # now ride ACT+Pool, so nothing interleaves into the k path)
            for j in range(NCT):
                nc.vector.tensor_mul(sqk[:, j, :], xk[:, j, :], xk[:, j, :])

            # ---- q4 projections ----
            q4 = P.tile([128, 2, LQ], BF16, tag="q4", name="q4")
            k4 = P.tile([128, 2, Lkp], BF16, tag="k4", name="k4")

            def emit_qk4_mains(nm, dt, o, s):
                ps = PSA.tile([128, 512], F32, tag="main", name="main")
                for j in range(NCT):
                    nc.tensor.matmul(ps[:, :s], w_bf[nm][:, j, dt * 128:(dt + 1) * 128],
                                     x_bf[nm][:, j, o:o + s], start=(j == 0), stop=False)
                return ps

            def emit_qk4_fin(nm, dt, o, s, ps):
                dest, rbc = (q4, rq_bc) if nm == "q" else (k4, None)
                st = stat[nm]
                nc.tensor.matmul(ps[:, :s], st["u_row"](o, dt * 128, 128),
                                 st["negmu_row"](o, s), start=False,
                                 stop=ln_identity)
                if not ln_identity:
                    nc.tensor.matmul(ps[:, :s], bw_sb[nm][:, dt * 128:(dt + 1) * 128],
                                     st["sd_row"](o, s), start=False, stop=True)
                if rbc is not None:
                    # ACT evicts the raw psum; the rstd multiply runs on the
                    # idle GPSIMD so the DVE queue stays clear for the k path
                    qraw = SQ.tile([128, 512], BF16, tag="qraw", name="qraw")
                    nc.scalar.activation(qraw[:, :s], ps[:, :s], AF.Copy)
                    nc.gpsimd.tensor_mul(dest[:, dt, o:o + s], qraw[:, :s],
                                         rbc[:, o:o + s])
                elif dt == 0 and o == 0:
                    # only the first (gate-critical) k4 eviction rides ACT;
                    # the second would displace the earliest exps
                    nc.scalar.activation(dest[:, dt, o:o + s], ps[:, :s], AF.Copy)
                else:
                    nc.vector.tensor_copy(dest[:, dt, o:o + s], ps[:, :s])

            def emit_qk4(nm, dt, o, s):
                emit_qk4_fin(nm, dt, o, s, emit_qk4_mains(nm, dt, o, s))

            emit_qk4("q", 0, 0, 512)
            emit_qk4("q", 0, 512, 512)
            emit_qk4("q", 1, 0, 512)
            emit_qk4("q", 1, 512, 512)

            emit_stats("k", Lkp)
            rk_col = rstd_of("k", SCALE)

            # ---- attention pipeline ----
            groups = [(o, s, dt) for (o, s) in _slices(LQ, 512) for dt in range(2)]
            et_of = {}

            ep_ctx = tc.tile_pool(name="epool", bufs=6)
            EP = ep_ctx.__enter__()

            def emit_sexp(g, kts=None):
                (o, s, dt) = groups[g]
                if kts is None:
                    kts = range(NKT)
                if g in et_of:
                    ets = et_of[g]
                else:
                    ets = []
                    for hh in range(2):
                        et = EP.tile([128, NKT, 512], BF16, tag="E", name="E")
                        ets.append(et)
                    et_of[g] = ets
                for kt in kts:
                    pss = []
                    for hh in range(2):
                        pb = 64 * hh
                        ps = PSS.tile([128, 512], F32, tag="S", name="S")
                        pss.append(ps)
                        nc.tensor.matmul(ps[:, :s],
                                         k4[pb:pb + CH, dt, kt * 128:(kt + 1) * 128],
                                         q4[pb:pb + CH, dt, o:o + s],
                                         start=True, stop=True)
                    for hh in range(2):
                        nc.scalar.activation(ets[hh][:, kt, :s], pss[hh][:, :s],
                                             AF.Exp,
                                             bias=madd_sb[:, kt:kt + 1],
                                             scale=rk_col[:, kt:kt + 1])

            # interleave k4 eviction halves with the S/exp k-tiles they gate;
            # the v squares ride the dt0 stretch on DVE, and the v stats sit
            # just inside dt1 so Ln_v/Exp_rv land at the g0/g1 boundary of
            # the in-order ACT exp queue
            rv_col = None
            for dt in range(2):
                if dt == 1:
                    emit_qk4("k", 1, 0, min(512, Lkp))
                    emit_stats("v", Lkp)
                    rv_col = rstd_of("v")
                for i, (o, s) in enumerate(_slices(Lkp, 512)):
                    if dt != 1 or o != 0:
                        emit_qk4("k", dt, o, s)
                    if dt == 0:
                        hi = NCT if o + s >= Lkp else min(4 * i + 4, NCT)
                        for j in range(4 * i, hi):
                            nc.vector.tensor_mul(sqv[:, j, :], xv[:, j, :],
                                                 xv[:, j, :])
                    emit_sexp(dt, range(o // 128, (o + s) // 128))

            # ---- v4a: [128t, HPC*(CH+1)] per ktile; 65th col = ones ----
            v4a = P.tile([128, NKT, HPC * (CH + 1)], BF16, tag="v4a", name="v4a")

            def emit_v4a_kt(kt):
                ps = PSA.tile([128, 512], F32, tag="main", name="main")
                for j in range(NCT):
                    nc.tensor.matmul(ps[:, :HC], x_bf["v"][:, j, kt * 128:(kt + 1) * 128],
                                     w_bf["v"][:, j, :], start=(j == 0), stop=False)
                nc.tensor.matmul(ps[:, :HC],
                                 stat["v"]["negmu_row"](kt * 128, 128),
                                 stat["v"]["u_row"](kt * 128, 0, HC), start=False,
                                 stop=ln_identity)
                if not ln_identity:
                    nc.tensor.matmul(ps[:, :HC],
                                     stat["v"]["sd_row"](kt * 128, 128),
                                     bw_sb["v"][:, :], start=False, stop=True)
                nc.vector.tensor_scalar_mul(
                    v4a[:, kt, :].rearrange("p (h x) -> p h x", h=HPC)[:, :, 0:CH],
                    ps[:, 0:HC].rearrange("p (h x) -> p h x", h=HPC),
                    rv_col[:, kt:kt + 1])
                nc.vector.memset(
                    v4a[:, kt, :].rearrange("p (h x) -> p h x", h=HPC)[:, :, CH:CH + 1], 1.0)

            # oqc[q, qt, h, c]: normalized per-head attention output, q-major
            oqc = P.tile([128, NQT, HPC, CH], BF16, tag="oqc", name="oqc")
            o_sb = P.tile([128, HPC // 2, LQ], BF16, tag="o_sb", name="o_sb")

            def emit_pv_qt(g, qt):
                (o, s, dt) = groups[g]
                ets = et_of[g]
                qo = qt * 128 - o
                for hh in range(2):
                    h = 2 * dt + hh
                    et = ets[hh]
                    ppv = PSPV.tile([128, CH + 1], F32, tag="pv", name="pv")
                    for kt in range(NKT):
                        nc.tensor.matmul(ppv[:, :],
                                         et[:, kt, qo:qo + 128],
                                         v4a[:, kt, h * (CH + 1):(h + 1) * (CH + 1)],
                                         start=(kt == 0), stop=(kt == NKT - 1))
                    rcp = R2.tile([128, 1], F32, tag="rcp", name="rcp", bufs=8)
                    nc.vector.reciprocal_approx_fast(out=rcp, in_=ppv[:, CH:CH + 1])
                    nc.vector.tensor_scalar_mul(oqc[:, qt, h, :],
                                                ppv[:, 0:CH], rcp)
                if dt == 1:
                    eng = nc.scalar if g == 3 else nc.sync
                    for t in range(HPC // 2):
                        # g3 transposes issue from ACT (idle post-exp, and
                        # its SEQ has no pending out-DMA waits like SP)
                        eng.dma_start_transpose(
                            out=o_sb[:, t, qt * 128:(qt + 1) * 128],
                            in_=oqc[:, qt, 2 * t:2 * t + 2, :])
                if qt == (o + s) // 128 - 1:
                    et_of.pop(g)

            def emit_proj(o, s, tail=False):
                for dt2 in range(NDT):
                    ps = PSA.tile([128, 512], F32, tag="main", name="main")
                    for t in range(HPC // 2):
                        nc.tensor.matmul(ps[:, :s], wp_bf[:, t, dt2 * 128:(dt2 + 1) * 128],
                                         o_sb[:, t, o:o + s], start=(t == 0),
                                         stop=(t == HPC // 2 - 1))
                    ot = SQ.tile([128, 512], BF16, tag="ot", name="ot")
                    if bproj_zero and tail:
                        # halve each eviction across DVE+ACT so the PSA
                        # rotation isn't paced by one serial DVE stream
                        h = s // 2
                        nc.vector.tensor_copy(ot[:, :h], ps[:, :h])
                        nc.scalar.activation(ot[:, h:s], ps[:, h:s], AF.Copy)
                    elif bproj_zero:
                        nc.vector.tensor_copy(ot[:, :s], ps[:, :s])
                    else:
                        nc.vector.tensor_scalar_add(ot[:, :s], ps[:, :s],
                                                    bproj_sb[:, dt2:dt2 + 1])
                    nc.sync.dma_start(out=out[dt2 * 128:(dt2 + 1) * 128, o:o + s],
                                      in_=ot[:, :s])

            # ---- v4a, then drain. PV(g0)/PV(g1) run BEFORE the S(g2)/S(g3)
            # stretches: the in-order PE would otherwise sit inside the
            # exp-paced S lockstep (PSS rotation) while ready PV work waits.
            for kt in range(NKT):
                emit_v4a_kt(kt)
                if kt >= 2:
                    # feed S(g2) into the ACT exp queue while v4a runs
                    emit_sexp(2, [kt - 2])
            for qt in range(4):
                emit_pv_qt(0, qt)
            emit_sexp(2, [NKT - 2, NKT - 1])
            for qt in range(4):
                emit_pv_qt(1, qt)
            emit_sexp(3)
            emit_proj(0, 512)
            for qt in range(4, 8):
                emit_pv_qt(2, qt)
            for qt in range(4, 8):
                emit_pv_qt(3, qt)
            emit_proj(512, 512, tail=True)
            ep_ctx.__exit__(None, None, None)
            sqp_ctx.__exit__(None, None, None)

    _compile_pinned(nc)
    return nc


def prepare_in_maps(q, k, v, qpos, kpos, mask,
                    ln_q_w, ln_q_b, ln_k_w, ln_k_b, ln_v_w, ln_v_b,
                    w_q, w_k, w_v, w_proj, b_proj):
    import ml_dtypes
    bf = ml_dtypes.bfloat16
    f = np.float32
    q = np.asarray(q, f) + np.asarray(qpos, f).reshape(B, LQ, C)
    k = np.asarray(k, f) + np.asarray(kpos, f).reshape(B, LK, C)
    v = np.asarray(v, f)
    mask = np.asarray(mask)

    keeps = [np.flatnonzero(mask[b, 0, 0] == 0) for b in range(B)]
    Lkp = max(128, -(-max(len(kp) for kp in keeps) // 128) * 128)
    NKT = Lkp // 128

    def colmajor(vec, ntiles):
        return np.ascontiguousarray(vec.reshape(ntiles, 128).T.astype(f))

    ident = all(np.all(np.asarray(g) == 1.0) for g in (ln_q_w, ln_k_w, ln_v_w)) \
        and all(np.all(np.asarray(b) == 0.0) for b in (ln_q_b, ln_k_b, ln_v_b))
    bz = bool(np.all(np.asarray(b_proj) == 0.0))

    # gamma folds into W host-side; beta enters as host-computed W@beta rows
    w_eff = {}
    bw_full = {}
    for nm, w_, g_, b_ in (("q", w_q, ln_q_w, ln_q_b), ("k", w_k, ln_k_w, ln_k_b),
                           ("v", w_v, ln_v_w, ln_v_b)):
        w_ = np.asarray(w_, f)
        if ident:
            w_eff[nm] = w_
        else:
            w_eff[nm] = w_ * np.asarray(g_, f)[None, :]
            bw_full[nm] = w_ @ np.asarray(b_, f)

    in_maps = []
    for core in range(8):
        b, hg = core // 4, core % 4
        kp = keeps[b]
        nk = len(kp)
        hs = slice(hg * HC, (hg + 1) * HC)

        def padT(x2d):  # [n, C] -> [C, Lkp] bf16
            outp = np.zeros((C, Lkp), bf)
            outp[:, :x2d.shape[0]] = x2d.T.astype(bf)
            return np.ascontiguousarray(outp)

        madd_np = np.full(Lkp, -1e30, f)
        madd_np[:nk] = 0.0
        m = {
            "qT": np.ascontiguousarray(q[b].T.astype(bf)),
            "kT": padT(k[b][kp]),
            "vT": padT(v[b][kp]),
            "wqT": np.ascontiguousarray(w_eff["q"][hs, :].T.astype(bf)),
            "wkT": np.ascontiguousarray(w_eff["k"][hs, :].T.astype(bf)),
            "wvT": np.ascontiguousarray(w_eff["v"][hs, :].T.astype(bf)),
            # wp[64*(h%2)+p, h//2, d] = w_proj[d, hg*256 + 64h + p]
            "wp": np.ascontiguousarray(
                np.asarray(w_proj, f)[:, hs].T.reshape(HPC // 2, 2, CH, C)
                .transpose(1, 2, 0, 3).reshape(128, HPC // 2, C).astype(bf)),
            "madd": colmajor(madd_np, NKT),
            "eye": np.ascontiguousarray(np.eye(128).astype(bf)),
        }
        if not ident:
            m["bwq"] = np.ascontiguousarray(bw_full["q"][hs][None, :].astype(bf))
            m["bwk"] = np.ascontiguousarray(bw_full["k"][hs][None, :].astype(bf))
            m["bwv"] = np.ascontiguousarray(bw_full["v"][hs][None, :].astype(bf))
        if not bz:
            m["bproj"] = colmajor(
                np.asarray(b_proj, f) if hg == 0 else np.zeros(C, f), NDT)
        in_maps.append(m)
    return in_maps, Lkp, ident, bz


def kernel(**inputs):
    global LAST_EXEC_NS, LAST_RESULTS
    f = np.float32
    in_maps, Lkp, ident, bz = prepare_in_maps(**inputs)
    key = (Lkp, ident, bz)
    nc = _NC_CACHE.get(key)
    if nc is None:
        nc = build_nc(Lkp, ln_identity=ident, bproj_zero=bz)
        _NC_CACHE[key] = nc
    trace = os.environ.get("KERNEL_TRACE", "0") == "1"
    res = run_bass_kernel_spmd(nc, in_maps, core_ids=list(range(8)), trace=trace)
    LAST_EXEC_NS = res.exec_time_ns
    LAST_RESULTS = res

    out_full = np.zeros((B, LQ, C), f)
    for b in range(B):
        acc = np.zeros((C, LQ), f)
        for hg in range(4):
            acc += res.results[b * 4 + hg]["out"].astype(f)
        out_full[b] = acc.T
    return out_full
